# revision 10
# baseline (speedup 1.0000x reference)
"""Trainium2 Bass kernel: BiDAF-style attention (B=32, C=1024, Q=128, d=768).

Data-parallel over batch: 4 batches per NeuronCore x 8 cores, no collectives.

Math (per batch b):
  sim[c,q]  = x_qc[c,q] + x_c[c] + x_q[q],  x_qc = ctx @ (query*wqc)^T
  P[:,c]    = softmax_q(sim[c,:])   -> c2q = P^T-ish matmul with query
  q2c_w     = softmax_c(max_q sim)  -> q2c = q2c_w @ ctx
  g = [ctx, c2q, ctx*c2q, ctx*q2c]

Device-side restructuring:
  - simT kept in [q, c] layout; exp(simT + x_q) fused on ScalarE (bias AP).
    exp(x_c) cancels in the q-softmax; it is re-applied only on the tiny
    [C]-sized q2c path (host ships exp(x_c)).
  - c2q = E^T @ [query | 1]: the ones column gives the softmax denominator
    for free; normalization is fused into the PSUM evacuation (ScalarE
    scale) and into the g3 multiply (scalar_tensor_tensor on VectorE).
  - max_q E per c-block via PE transpose + free-dim reduce_max.
  - q2c row is broadcast across partitions with a DRAM-bounce broadcast DMA.
"""

import os

# The device run goes through jax's axon PJRT backend. If the calling
# process pinned JAX_PLATFORMS (e.g. to "cpu" for a reference run), make
# sure axon is still visible and preferred.
_jp = os.environ.get("JAX_PLATFORMS")
if _jp is not None and "axon" not in _jp.split(","):
    os.environ["JAX_PLATFORMS"] = "axon," + _jp

import numpy as np

B, C, Q, D = 32, 1024, 128, 768
N_CORES = 8
BPC = B // N_CORES          # batches per core
CBLK = C // 128             # 8 c-blocks of 128
DBLK = D // 128             # 6 d-blocks of 128
QAUG = D + 2                # 770 free cols: [c2q | denom | pad] (even chunks for fp32r)

# Use reduced-precision fp32 (float32r: single-pass PE matmul, full rate at
# N>=256) for the sim and c2q matmuls. The tensors feeding those matmuls are
# declared float32r end-to-end (same 4-byte fp32 bits; the PE rounds
# internally). Flip with env KBENCH_FP32R=0 for exact fp32 (4 cycles/row).
_FP32R = os.environ.get("KBENCH_FP32R", "1") == "1"

LAST_RESULT = None  # BassKernelResults of the most recent device run

# This toolchain's walrus embeds at most one sync wait per engine
# instruction; Tile freely attaches several. Hoist extras onto standalone
# EventSemaphore carriers inserted just before the instruction on the same
# engine — sequencers process their stream in order, so the carrier gates
# everything after it.
_MAX_EMBEDDED_WAITS = 1


def _split_waits(nc):
    import concourse.mybir as mybir

    n = 0
    for f in nc.m.functions:
        for blk in f.blocks:
            new_insts = []
            for inst in blk.instructions:
                si = inst.sync_info
                waits = list(si.on_wait) if si is not None else []
                if len(waits) > _MAX_EMBEDDED_WAITS:
                    keep = waits[-_MAX_EMBEDDED_WAITS:]
                    for w in waits[: len(waits) - _MAX_EMBEDDED_WAITS]:
                        ev = mybir.InstEventSemaphore(
                            name=f"{inst.name}-wsplit{n}", ins=[], outs=[]
                        )
                        ev.engine = inst.engine
                        ev.sync_info = mybir.SyncInfo(on_wait=[w], on_update=[])
                        new_insts.append(ev)
                        n += 1
                    inst.sync_info = mybir.SyncInfo(
                        on_wait=keep, on_update=list(si.on_update)
                    )
                new_insts.append(inst)
            blk.instructions = new_insts
    return n


def build_bass(sim=False):
    """Build the per-core Bass/Tile program. Same program on all 8 cores."""
    from contextlib import ExitStack

    import concourse.bass as bass
    import concourse.tile as tile
    from concourse import mybir

    f32 = mybir.dt.float32
    f32m = mybir.dt.float32r if _FP32R else mybir.dt.float32
    AF = mybir.ActivationFunctionType
    MULT = mybir.AluOpType.mult
    AX = mybir.AxisListType.X

    if sim:
        from concourse import bacc

        nc = bacc.Bacc(None, target_bir_lowering=False, debug=True)
    else:
        nc = bass.Bass()

    ctx_d = nc.declare_dram_parameter("ctx", [BPC, C, D], f32, isOutput=False)
    ctxT_d = nc.declare_dram_parameter("ctxT", [BPC, D, C], f32m, isOutput=False)
    qwT_d = nc.declare_dram_parameter("qwT", [BPC, D, Q], f32m, isOutput=False)
    qaug_d = nc.declare_dram_parameter("qaug", [BPC, Q, QAUG], f32m, isOutput=False)
    xq_d = nc.declare_dram_parameter("xq", [Q, BPC], f32, isOutput=False)
    exc_d = nc.declare_dram_parameter("exc", [128, BPC, CBLK], f32, isOutput=False)
    ident_d = nc.declare_dram_parameter("ident", [128, 128], f32m, isOutput=False)
    g_d = nc.declare_dram_parameter("g", [BPC, C, 4 * D], f32, isOutput=True)

    with tile.TileContext(nc) as tc, ExitStack() as es:
        singles = es.enter_context(tc.tile_pool(name="singles", bufs=1))
        big = es.enter_context(tc.tile_pool(name="big", bufs=2))
        epool = es.enter_context(tc.tile_pool(name="epool", bufs=2))
        stg_pool = es.enter_context(tc.tile_pool(name="stg", bufs=3))
        small = es.enter_context(tc.tile_pool(name="small", bufs=4))
        bc_pool = es.enter_context(tc.tile_pool(name="bcast", bufs=2))
        dram = es.enter_context(tc.tile_pool(name="dram", bufs=2, space="DRAM"))
        ps_sim = es.enter_context(tc.tile_pool(name="ps_sim", bufs=2, space="PSUM"))
        ps_c2q = es.enter_context(tc.tile_pool(name="ps_c2q", bufs=1, space="PSUM"))
        ps_et = es.enter_context(tc.tile_pool(name="ps_et", bufs=2, space="PSUM"))
        ps_q2c = es.enter_context(tc.tile_pool(name="ps_q2c", bufs=1, space="PSUM"))

        identity = singles.tile([128, 128], f32m)
        nc.sync.dma_start(identity, ident_d[:, :])
        ones_col = singles.tile([128, 1], f32)
        nc.vector.memset(ones_col, 1.0)
        xq_t = singles.tile([Q, BPC], f32)
        nc.sync.dma_start(xq_t, xq_d[:, :])
        exc_t = singles.tile([128, BPC, CBLK], f32)
        nc.sync.dma_start(exc_t, exc_d[:, :, :])

        for b in range(BPC):
            ctx_t = big.tile([128, CBLK, D], f32, tag="ctx")
            nc.sync.dma_start(ctx_t, ctx_d[b].rearrange("(blk p) d -> p blk d", p=128))
            ctxT_t = big.tile([128, DBLK, C], f32m, tag="ctxT")
            nc.sync.dma_start(ctxT_t, ctxT_d[b].rearrange("(k p) c -> p k c", p=128))
            qwT_t = big.tile([128, DBLK, Q], f32m, tag="qwT")
            nc.sync.dma_start(qwT_t, qwT_d[b].rearrange("(k p) q -> p k q", p=128))
            qaug_t = big.tile([Q, QAUG], f32m, tag="qaug")
            nc.sync.dma_start(qaug_t, qaug_d[b])

            # ---- simT[q, c] = (query*wqc) @ ctx^T, then E = exp(simT + x_q)
            E_t = epool.tile([Q, C], f32m, tag="E")
            for half in range(2):
                sim_ps = ps_sim.tile([Q, 512], f32, tag="sim")
                for k in range(DBLK):
                    nc.tensor.matmul(
                        sim_ps,
                        lhsT=qwT_t[:, k, :],
                        rhs=ctxT_t[:, k, half * 512 : (half + 1) * 512],
                        start=(k == 0),
                        stop=(k == DBLK - 1),
                    )
                nc.scalar.activation(
                    E_t[:, half * 512 : (half + 1) * 512],
                    sim_ps,
                    AF.Exp,
                    bias=xq_t[:, b : b + 1],
                    scale=1.0,
                )

            # ---- q2c path: m[c] = max_q E (via PE transpose), weight = m*exc
            m_t = small.tile([128, CBLK], f32, tag="m")
            for blk in range(CBLK):
                et_ps = ps_et.tile([128, 128], f32m, tag="et")
                nc.tensor.transpose(et_ps, E_t[:, blk * 128 : (blk + 1) * 128], identity)
                nc.vector.reduce_max(m_t[:, blk : blk + 1], et_ps, axis=AX)

            m2_t = small.tile([128, CBLK], f32, tag="m2")
            nc.vector.tensor_mul(m2_t, m_t, exc_t[:, b, :])
            msum_t = small.tile([128, 1], f32, tag="msum")
            nc.vector.reduce_sum(msum_t, m2_t, axis=AX)

            q2c_ps = ps_q2c.tile([1, QAUG], f32)
            for lo, hi in ((0, 512), (512, 768)):
                for blk in range(CBLK):
                    nc.tensor.matmul(
                        q2c_ps[:, lo:hi],
                        lhsT=m2_t[:, blk : blk + 1],
                        rhs=ctx_t[:, blk, lo:hi],
                        start=(blk == 0),
                        stop=(blk == CBLK - 1),
                    )
            nc.tensor.matmul(
                q2c_ps[:, D : D + 1], lhsT=ones_col, rhs=msum_t, start=True, stop=True
            )

            zr_t = small.tile([1, 1], f32, tag="zr")
            nc.vector.reciprocal(zr_t, q2c_ps[:, D : D + 1])
            q2c_sb = small.tile([1, D], f32, tag="q2c")
            nc.scalar.mul(q2c_sb, q2c_ps[:, 0:D], zr_t)

            # broadcast q2c row to 128 partitions via DRAM bounce
            q2c_dram = dram.tile([1, D], f32)
            nc.gpsimd.dma_start(q2c_dram, q2c_sb)
            bcast_t = bc_pool.tile([128, D], f32, tag="bc")
            q2c_ap = q2c_dram[:, :]
            nc.gpsimd.dma_start(
                bcast_t,
                bass.AP(tensor=q2c_ap.tensor, offset=q2c_ap.offset, ap=[[0, 128], [1, D]]),
            )

            # ---- c2q per c-block + output assembly
            for blk in range(CBLK):
                c2q_ps = ps_c2q.tile([128, QAUG], f32)
                for lo, hi in ((0, 512), (512, QAUG)):
                    nc.tensor.matmul(
                        c2q_ps[:, lo:hi],
                        lhsT=E_t[:, blk * 128 : (blk + 1) * 128],
                        rhs=qaug_t[:, lo:hi],
                        start=True,
                        stop=True,
                    )
                rs_t = small.tile([128, 1], f32, tag="rs")
                nc.vector.reciprocal(rs_t, c2q_ps[:, D : D + 1])

                stg = stg_pool.tile([128, 3 * D], f32, tag="stg")
                # g2 = c2q (normalized) -- ScalarE evac with fused scale
                nc.scalar.mul(stg[:, 0:D], c2q_ps[:, 0:D], rs_t)
                # g3 = ctx * c2q -- fused (psum * rs) * ctx on VectorE
                nc.vector.scalar_tensor_tensor(
                    stg[:, D : 2 * D],
                    in0=c2q_ps[:, 0:D],
                    scalar=rs_t,
                    in1=ctx_t[:, blk, :],
                    op0=MULT,
                    op1=MULT,
                )
                # g4 = ctx * q2c
                nc.vector.tensor_mul(stg[:, 2 * D : 3 * D], ctx_t[:, blk, :], bcast_t)

                nc.scalar.dma_start(
                    g_d[b, blk * 128 : (blk + 1) * 128, D : 4 * D], stg
                )
                nc.gpsimd.dma_start(
                    g_d[b, blk * 128 : (blk + 1) * 128, 0:D],
                    ctx_t[:, blk, :],
                )

    if not sim:
        _split_waits(nc)
    return nc


def prepare_inputs(context, context_mask, query, query_mask, wq, wc, wqc):
    """Host-side prep: fold weights/masks, transpose, shard across 8 cores."""
    ctx = np.ascontiguousarray(np.asarray(context, dtype=np.float32))
    qry = np.ascontiguousarray(np.asarray(query, dtype=np.float32))
    cmask = np.asarray(context_mask)
    qmask = np.asarray(query_mask)
    wq = np.asarray(wq, dtype=np.float32)
    wc = np.asarray(wc, dtype=np.float32)
    wqc = np.asarray(wqc, dtype=np.float32)

    qw = qry * wqc[None, None, :]
    xq = np.einsum("bqd,d->bq", qry, wq).astype(np.float32)
    xc = np.einsum("bcd,d->bc", ctx, wc).astype(np.float32)
    # Mask folding: masked q -> -1e30 bias inside exp; masked c -> exc=0.
    xq_eff = np.where(qmask == 1, xq, np.float32(-1e30)).astype(np.float32)
    with np.errstate(over="ignore"):
        exc = np.exp(
            np.where(cmask == 1, xc, np.float32(-np.inf)), dtype=np.float32
        )

    ctxT = np.ascontiguousarray(ctx.transpose(0, 2, 1))
    qwT = np.ascontiguousarray(qw.transpose(0, 2, 1).astype(np.float32))
    qaug = np.concatenate(
        [qry, np.ones((B, Q, 1), np.float32), np.zeros((B, Q, 1), np.float32)],
        axis=2,
    )

    in_maps = []
    for i in range(N_CORES):
        sl = slice(i * BPC, (i + 1) * BPC)
        in_maps.append(
            {
                "ctx": ctx[sl],
                "ctxT": ctxT[sl],
                "qwT": qwT[sl],
                "qaug": np.ascontiguousarray(qaug[sl]),
                "xq": np.ascontiguousarray(xq_eff[sl].T),
                "exc": np.ascontiguousarray(
                    exc[sl].reshape(BPC, CBLK, 128).transpose(2, 0, 1)
                ),
                "ident": np.eye(128, dtype=np.float32),
            }
        )
    return in_maps


def kernel(context, context_mask, query, query_mask, wq, wc, wqc):
    global LAST_RESULT
    from concourse.bass_utils import run_bass_kernel_spmd

    in_maps = prepare_inputs(
        context, context_mask, query, query_mask, wq, wc, wqc
    )
    nc = build_bass()
    res = run_bass_kernel_spmd(nc, in_maps, core_ids=list(range(N_CORES)))
    LAST_RESULT = res
    out = np.concatenate([res.results[i]["g"] for i in range(N_CORES)], axis=0)
    return np.ascontiguousarray(out.reshape(B, C, 4 * D))
